# revision 34
# baseline (speedup 1.0000x reference)
"""NT-Xent contrastive loss (SimCLR-style) on 8 Trainium2 NeuronCores.

Problem: z1, z2 [4096, 256] fp32 -> scalar loss.
  zn = l2norm(z), z = concat(z1, z2) -> [8192, 256]
  sim = zn @ zn.T / 0.07              -> [8192, 8192]
  loss = -mean_i log_softmax(sim)[i, partner(i)],  partner(i) = (i + 4096) % 8192

Strategy (symmetric): exp(sim) is symmetric, so each unordered tile pair
{a, b} of the 64x64 grid of 128x128 blocks is computed ONCE. The core
owning row-tile a computes blocks (a, a+o mod 64) for o = 0..32 (the
o=32 pair is computed by both owners: 3% redundancy that keeps the
program SPMD-identical). Row sums come for free from the DVE via
tensor_scalar's accum_out (runs in the 4x perf mode on packed bf16, so
~1.1us per row tile instead of tensor_reduce's 4.5us); the transpose
credit for o = 1..31 comes from COLUMN sums of the same exp block,
computed on the PE as ones^T @ E matmuls accumulating in PSUM. Host
adds the per-core partial sums.

Per-core input is the normalized z^T rolled so its own 1024 rows sit at
columns [0, 1024): every core runs one identical program, and the o-arcs
become contiguous column ranges [0, 5120) -- only 62.5% of z is even
loaded. Matmuls run in fp8e4m3 (values pre-scaled x16) with DoubleRow
packing K=256 into one pass; exp runs on the scalar engine PSUM->SBUF in
bf16. Tolerance is rel 2e-2; fp8 error lands ~1e-3.

exp SBUF layout per q: [o0 | o1 | ... | o31 | o32] (4096 + 128 = 4224),
so gram blocks are clean 1024-wide slabs [qs+1024b, qs+1024(b+1)) that
align with the z DMA chunks, and the o=32 tail (which needs the last z
columns) is emitted LAST -- the first matmul only waits on the first
1024-column DMA chunk. Column-sum matmuls are interleaved between gram
blocks as PE filler so the tensor engine stays busy (and stays out of
the low p-states) while ACT drains the previous block.

PSUM discipline: an accumulation group conflicts with any other group
opened in the same bank while it is live, so long-lived accumulators
get exclusive banks. Banks 0-3: gram double-buffer ([128,1024] x 2).
Banks 4-7: col-sum chunks cc=2..5 (target tiles 8..23), held open
across the whole q loop. Chunks cc=6,7 and the 7 inter-core edge tiles
(t=32..38) run at the tail through the drained banks; the 7 intra-core
edge tiles (t=1..7) run mid-loop as brief transient groups inside a
gram slot (their DVE drain is quick now that the DVE is nearly idle).
"""

import numpy as np

import concourse.bass as bass
import concourse.tile as tile
from concourse import bacc, mybir
from concourse.bass_utils import run_bass_kernel_spmd

B, D = 4096, 256
N = 2 * B            # 8192 embeddings
NCORES = 8
NT = N // 128        # 64 tiles of 128 embeddings
Q = 8                # row tiles per core
ARC = 33             # column tiles per row tile (o = 0..32)
COLS = ARC * 128     # 4224
OCOLS = 32 * 128     # 4096: o0..o31 region; o32 tail at [4096, 4224)
ZCOLS = (Q - 1 + 32 + 1) * 128   # 5120 rolled columns needed per core
TEMP = 0.07
ZSCALE = 16.0        # pre-scale before fp8 cast (keeps values in e4m3 normal range)
EXP_SCALE = 1.0 / (ZSCALE * ZSCALE * TEMP)

F32 = mybir.dt.float32
BF16 = mybir.dt.bfloat16
FP8 = mybir.dt.float8e4

EDGE_TILES = list(range(1, 8)) + list(range(32, 39))

# Rowsum strategy: the DVE reduce-with-accumulator paths all run at
# 1 el/cycle on hw (the 2x/4x packed modes don't apply to reductions), so
# a monolithic 4224-wide reduce costs 4.5us and serializes the pipeline.
# Split it instead: the B0 block's sum rides the ACT exp instruction via
# accum_out (~0.2-0.3us accumulator read), and B1, B2, B3+B4 get their
# own DVE tensor_reduce partials (~1.1us each, interleaved with drains).
# Host adds the 4 partials per row tile.
RS_PARTS = 4         # rowsum partials per q: [ACT B0, DVE B1, DVE B2, DVE B3+B4]

# z DMA column chunks, issued in ascending order across three engine
# sequencers so early gram matmuls only wait on the small first chunk.
ZCHUNKS = [(0, 512), (512, 1536), (1536, 2560), (2560, 3584), (3584, 4608),
           (4608, 5120)]


def build_nc() -> bass.Bass:
    nc = bacc.Bacc("TRN2", target_bir_lowering=False, debug=False, num_devices=NCORES)
    zdr = nc.declare_dram_parameter("zdr", [128, 2, ZCOLS], FP8, isOutput=False)
    ones_d = nc.declare_dram_parameter("ones", [128, 32], BF16, isOutput=False)
    rowsum_d = nc.declare_dram_parameter("rowsum", [128, RS_PARTS * Q], F32, isOutput=True)
    o32_d = nc.declare_dram_parameter("o32", [128, Q * 128], F32, isOutput=True)
    mid_d = nc.declare_dram_parameter("mid", [1, 6 * 512], F32, isOutput=True)
    edge_d = nc.declare_dram_parameter("edge", [1, 14 * 128], F32, isOutput=True)

    with tile.TileContext(nc) as tc:
        with (
            tc.tile_pool(name="zp", bufs=1) as zp,
            tc.tile_pool(name="const", bufs=1) as constp,
            tc.tile_pool(name="expp", bufs=8) as expp,
            tc.tile_pool(name="stats", bufs=1) as statsp,
            tc.tile_pool(name="gram", bufs=2, space="PSUM") as gramp,
            tc.tile_pool(name="cacc", bufs=1, space="PSUM") as caccp,
        ):
            # ACT exp-table preload on a zeroed tile, overlapping the z DMA
            zeros = statsp.tile([128, 512], F32)
            nc.any.memset(zeros[:], 0.0)
            warm = statsp.tile([32, 2], F32)
            nc.scalar.activation(
                warm[:, 0:1], zeros[0:32, 0:1], mybir.ActivationFunctionType.Exp
            )
            # PE p-state warm-up: the tensor engine reaches full clock only
            # after ~3us of continuous execution, so burn the DMA wait on
            # dummy matmuls over a zeroed tile (output never read)
            wz = statsp.tile([128, 512], BF16)
            nc.any.memset(wz[:], 0.0)
            wps = gramp.tile([128, 1024], F32, tag="g")
            for _ in range(6):
                nc.tensor.matmul(
                    wps[:, 0:512], lhsT=wz[:, 0:128], rhs=wz[:, 0:512],
                    start=True, stop=True,
                )

            # Spread the input DMA issue across idle engine sequencers: a
            # single sequencer spends ~620ns per dma_start dispatch, so 7
            # serial dispatches on sync would delay the last z chunk by 4us.
            z = zp.tile([128, 2, ZCOLS], FP8)
            ones = constp.tile([128, 32], BF16)
            issuers = [nc.sync, nc.scalar, nc.gpsimd, nc.sync, nc.scalar, nc.gpsimd]
            for (c0, c1), eng in zip(ZCHUNKS, issuers):
                eng.dma_start(z[:, :, c0:c1], zdr[:, :, c0:c1])
            nc.sync.dma_start(ones[:], ones_d[:])

            rowsum = statsp.tile([128, RS_PARTS * Q], F32)
            o32stage = statsp.tile([128, Q * 128], F32)
            edgestage = statsp.tile([32, 14 * 128], F32)
            midstage = statsp.tile([32, 6 * 512], F32)
            # long-lived col-sum accumulator banks (chunks cc=2..5; banks
            # reused at the tail for cc=6,7 and the t>=32 edge tiles)
            macc = [
                caccp.tile([128, 512], F32, tag=f"M{i}", name=f"macc{i}")
                for i in range(4)
            ]

            def gram_mm(out_ap, qs, c0, c1):
                """out = z[:, qs:qs+128].T @ z[:, c0:c1] (scaled x256)."""
                nc.tensor.matmul(
                    out_ap,
                    lhsT=z[:, :, qs:qs + 128],
                    rhs=z[:, :, c0:c1],
                    start=True,
                    stop=True,
                    perf_mode=mybir.MatmulPerfMode.DoubleRow,
                )

            def dve_copy(dst, src):
                # PSUM -> SBUF drain; only one non-scalar input may be PSUM
                p = src.shape[0]
                w = src.shape[-1]
                nc.vector.scalar_tensor_tensor(
                    out=dst,
                    in0=src,
                    scalar=0.0,
                    in1=zeros[0:p, 0:w],
                    op0=mybir.AluOpType.bypass,
                    op1=mybir.AluOpType.add,
                )

            ets = []

            # et offset of arc column x (x = rolled col - qs): et_off = x
            def colsum(granule, tq, x0, w, start, stop):
                nc.tensor.matmul(
                    granule,
                    lhsT=ones[:],
                    rhs=ets[tq][:, x0:x0 + w],
                    start=start,
                    stop=stop,
                )

            def mid_colsum(q2, cc):
                """Chunk cc (target tiles 4cc..4cc+3): PE accumulates across
                q2 in an exclusive PSUM bank."""
                colsum(
                    macc[cc - 2][0:32, 0:512], q2, 512 * cc - 128 * q2, 512,
                    start=(q2 == 0), stop=(q2 == Q - 1),
                )

            def emit_edge(t, e, granule):
                """All contributions for edge tile t, back-to-back (brief
                transient group), then drained to SBUF staging."""
                lo = max(0, t - 31)
                hi = min(7, t - 1)
                for q2 in range(lo, hi + 1):
                    colsum(
                        granule, q2, 128 * (t - q2), 128,
                        start=(q2 == lo), stop=(q2 == hi),
                    )
                dve_copy(edgestage[:, 128 * e:128 * (e + 1)], granule)

            for q in range(Q):
                qs = 128 * q
                et = expp.tile([128, OCOLS], BF16, tag="et")
                ets.append(et)
                # gram blocks B0..B3: 1024-wide slabs, 2x512 matmuls each,
                # exp'd as soon as written; col-sum filler interleaved so
                # the PE never idles waiting on ACT to free a PSUM slot
                # col-sum fillers go BEFORE each gram pair on the PE queue:
                # ready work must not sit behind a gram that's waiting on an
                # exp to free its PSUM slot (in-order head-of-line), and a
                # busy PE stays out of the slow p-states.
                pb = []
                for b in range(2):
                    if q >= 1:
                        mid_colsum(q - 1, 2 + b)
                    p = gramp.tile([128, 1024], F32, tag="g")
                    base = qs + 1024 * b
                    gram_mm(p[:, 0:512], qs, base, base + 512)
                    gram_mm(p[:, 512:1024], qs, base + 512, base + 1024)
                    pb.append(p)
                # B0's rowsum partial rides the exp via the ACT accumulator
                nc.scalar.activation(
                    et[:, 0:1024], pb[0][:],
                    mybir.ActivationFunctionType.Exp, scale=EXP_SCALE,
                    accum_out=rowsum[:, RS_PARTS * q:RS_PARTS * q + 1],
                )
                nc.scalar.activation(
                    et[:, 1024:2048], pb[1][:],
                    mybir.ActivationFunctionType.Exp, scale=EXP_SCALE,
                )
                for b in range(2, 4):
                    if q >= 1:
                        mid_colsum(q - 1, 2 + b)
                    p = gramp.tile([128, 1024], F32, tag="g")
                    base = qs + 1024 * b
                    gram_mm(p[:, 0:512], qs, base, base + 512)
                    gram_mm(p[:, 512:1024], qs, base + 512, base + 1024)
                    nc.scalar.activation(
                        et[:, 1024 * b:1024 * (b + 1)], p[:],
                        mybir.ActivationFunctionType.Exp, scale=EXP_SCALE,
                    )
                # C tile: the raw o=32 gram block [qs+4096, qs+4224) in its
                # first bank (drained to SBUF and shipped to the host, which
                # computes both its exp row-sum contribution and the
                # target-pair diagonal in fp64 -- no exp/diag work on
                # device), edge-tile accumulation group in its second bank.
                # Its readers are all quick DVE copies emitted BEFORE the big
                # reduces, so the slot recycles early for next q's B1 grams.
                ct = gramp.tile([128, 1024], F32, tag="g")
                gram_mm(ct[:, 0:128], qs, qs + 4096, qs + 4224)
                # both C-tile drains go before every reduce on the DVE queue:
                # they gate next q's B1 gram slot, the reduces gate nothing
                dve_copy(o32stage[:, 128 * q:128 * (q + 1)], ct[:, 0:128])
                if q >= 1:
                    # intra-core edge tile t=q needs ets[0..q-1] only
                    emit_edge(q, q - 1, ct[0:32, 512:640])
                for j, (x0, x1) in enumerate(
                    [(1024, 2048), (2048, 3072), (3072, 4096)], start=1
                ):
                    nc.vector.tensor_reduce(
                        out=rowsum[:, RS_PARTS * q + j:RS_PARTS * q + j + 1],
                        in_=et[:, x0:x1],
                        axis=mybir.AxisListType.X,
                        op=mybir.AluOpType.add,
                    )

            for cc in range(2, 6):
                mid_colsum(Q - 1, cc)

            # tail: cc=6,7 run through fresh gram-pool tiles (their banks
            # are free after the last exp) so they don't wait on the cc=2..5
            # drains; the 7 inter-core edge tiles round-robin over all 4
            # freed macc banks so the PE streams while the DVE drains trail
            for cc in range(2, 6):
                dve_copy(
                    midstage[:, 512 * (cc - 2):512 * (cc - 1)],
                    macc[cc - 2][0:32, 0:512],
                )
            for cc in (6, 7):
                gt = gramp.tile([128, 1024], F32, tag="g")
                gran = gt[0:32, 0:512]
                for q2 in range(Q):
                    colsum(
                        gran, q2, 512 * cc - 128 * q2, 512,
                        start=(q2 == 0), stop=(q2 == 7),
                    )
                dve_copy(midstage[:, 512 * (cc - 2):512 * (cc - 1)], gran)
            for e, t in enumerate(EDGE_TILES):
                if t < 32:
                    continue   # done mid-loop
                gran = macc[e % 4][0:32, 0:128]
                emit_edge(t, e, gran)

            # outputs fan out across sequencers so the 4 dispatches overlap
            nc.sync.dma_start(edge_d[:], edgestage[0:1, :])
            nc.scalar.dma_start(mid_d[:], midstage[0:1, :])
            nc.gpsimd.dma_start(rowsum_d[:], rowsum[:])
            nc.sync.dma_start(o32_d[:], o32stage[:])

    nc.compile()
    return nc


def make_in_maps(z1: np.ndarray, z2: np.ndarray) -> list[dict]:
    z = np.concatenate([np.asarray(z1), np.asarray(z2)], axis=0).astype(np.float64)
    zn = z / np.maximum(np.linalg.norm(z, axis=1, keepdims=True), 1e-12)
    ztn = np.ascontiguousarray(zn.T * ZSCALE)  # [256, 8192]
    zdt = mybir.dt.np(FP8)
    onesm = np.ones((128, 32), dtype=mybir.dt.np(BF16))
    in_maps = []
    for c in range(NCORES):
        rolled = np.roll(ztn, -1024 * c, axis=1)[:, :ZCOLS]
        # DoubleRow layout: [partition p, ko, x] = row (128*ko + p)
        zdr = np.ascontiguousarray(
            rolled.reshape(2, 128, ZCOLS).transpose(1, 0, 2)
        ).astype(zdt)
        in_maps.append({"zdr": zdr, "ones": onesm})
    return in_maps


def assemble(results: list[dict]) -> np.float32:
    S = np.zeros(N, dtype=np.float64)
    tgt_all = np.zeros(N, dtype=np.float64)
    for c in range(NCORES):
        r = results[c]
        rowsum = (
            r["rowsum"].astype(np.float64).reshape(128, Q, RS_PARTS).sum(axis=-1)
        )
        # raw o=32 gram blocks [p, q*128+j]: exp row-sums and the
        # target-pair diagonal both come out in fp64 here
        o32 = r["o32"].astype(np.float64).reshape(128, Q, 128)
        rowsum += np.exp(o32 * EXP_SCALE).sum(axis=-1)
        tgt = o32[np.arange(128), :, np.arange(128)]  # [128, Q]
        mid = r["mid"].astype(np.float64).reshape(6, 512)
        edge = r["edge"].astype(np.float64).reshape(14, 128)
        base = 1024 * c
        for q in range(Q):
            S[base + 128 * q: base + 128 * (q + 1)] += rowsum[:, q]
            tgt_all[base + 128 * q: base + 128 * (q + 1)] = tgt[:, q]
        for cc in range(2, 8):
            gidx = (512 * cc + np.arange(512) + base) % N
            S[gidx] += mid[cc - 2]
        for e, t in enumerate(EDGE_TILES):
            gidx = (128 * t + np.arange(128) + base) % N
            S[gidx] += edge[e]
    loss = np.mean(np.log(S) - tgt_all / (ZSCALE * ZSCALE * TEMP))
    return np.float32(loss)


_NC_CACHE: list = []


def kernel(z1: np.ndarray, z2: np.ndarray) -> np.ndarray:
    in_maps = make_in_maps(z1, z2)
    if not _NC_CACHE:
        _NC_CACHE.append(build_nc())
    nc = _NC_CACHE[0]
    res = run_bass_kernel_spmd(nc, in_maps, list(range(NCORES)))
    return assemble(res.results)


if __name__ == "__main__":
    rng = np.random.default_rng(0)
    z1 = rng.standard_normal((B, D), dtype=np.float32)
    z2 = rng.standard_normal((B, D), dtype=np.float32)
    print(kernel(z1, z2))


# revision 82
# speedup vs baseline: 1.4513x; 1.4513x over previous
"""NT-Xent contrastive loss (SimCLR-style) on 8 Trainium2 NeuronCores.

Problem: z1, z2 [4096, 256] fp32 -> scalar loss.
  zn = l2norm(z), z = concat(z1, z2) -> [8192, 256]
  sim = zn @ zn.T / 0.07              -> [8192, 8192]
  loss = -mean_i log_softmax(sim)[i, partner(i)],  partner(i) = (i + 4096) % 8192

Strategy (symmetric): exp(sim) is symmetric, so each unordered tile pair
{a, b} of the 64x64 grid of 128x128 blocks is computed ONCE. The core
owning row-tile a computes blocks (a, a+o mod 64) for o = 0..32 (the
o=32 pair is computed by both owners: 3% redundancy that keeps the
program SPMD-identical). The transpose credit for o = 1..31 comes from
COLUMN sums of the exp blocks, computed on the PE as ones^T @ E matmuls
accumulating in PSUM. Host adds the per-core partial sums.

Per-core input is the normalized z^T rolled so its own 1024 rows sit at
columns [0, 1024): every core runs one identical program, and the o-arcs
become contiguous column ranges [0, 5120) -- only 62.5% of z is even
loaded. Matmuls run in fp8e4m3 (values pre-scaled x16) with DoubleRow
packing K=256 into one pass; exp runs on the scalar (ACT) engine
PSUM->SBUF in bf16. Tolerance is rel 2e-2; fp8 error lands ~1e-3.

The steady-state loop is ACT-bound (4x 1024-wide exp per row tile,
~4.0us) and runs with ZERO gaps on the ACT queue -- pure back-to-back
exps, no accumulator reads. That requires exactly 4 gram-block writes
and 4 exp reads per q through the 2 PSUM gram slots -- the classic
double buffer where each write lands while the other slot's exp
executes. Everything that would break the 4-allocation rhythm is
evicted from the loop:
  - rowsum: B0/B2/B3 are DVE tensor_reduce partials (the DVE
    reduce-with-accumulate ISA paths all run 1 el/cycle on hw, so a
    monolithic 4096-wide reduce would serialize everything); B1 folds
    1024->256 with two elementwise adds on the otherwise-idle gpsimd
    (Pool supports SBUF tensor_tensor but not reduce-accumulate ops)
    before a short DVE reduce; host adds the 4 partials.
  - the o=32 tile: its RAW gram block is computed at the tail, drained
    to SBUF, and shipped to the host, which computes both its exp
    row-sums and the target-pair diagonal (log-softmax numerator) in
    fp64. No exp, no diag mask, no 5th activation on device.
  - edge-tile col-sums (t=1..7 and 32..38, the partial-overlap chunks)
    all run at the tail.
exp SBUF layout per q: [o0 | ... | o31] (4096 cols), 1024-aligned with
both the PSUM banks and the z DMA chunks, which are issued in ascending
column order across three engine sequencers (one sequencer needs ~620ns
per dma_start dispatch, so serial issue would delay the last chunk 4us).
Col-sum matmuls are emitted BEFORE each gram pair: ready PE work must
not queue behind a gram that waits on an exp (in-order head-of-line),
and a busy PE stays out of the slow p-states. Dummy warm-up matmuls
during the input DMA ramp the PE clock (full speed needs ~3us of
continuous execution).

PSUM discipline: an accumulation group conflicts with any other group
in the same bank while it is live, and the tile framework serializes
groups per TILE, so long-lived accumulators get exclusive single-bank
tiles. Banks 0-3: gram double-buffer ([128,1024] x 2). Banks 4-7:
col-sum chunks cc=2..5 (target tiles 8..23), held open across the whole
q loop. At the tail, chunks cc=6,7 run through fresh gram-pool tiles,
then the 8 o=32 blocks and 14 edge tiles interleave over a 6-slot bank
rotation with drains split across DVE and ACT; edge tiles go largest-
first so the final output DMA is gated only by the cheapest drains.
"""

import numpy as np

import concourse.bass as bass
import concourse.tile as tile
from concourse import bacc, mybir
from concourse.bass_utils import run_bass_kernel_spmd

B, D = 4096, 256
N = 2 * B            # 8192 embeddings
NCORES = 8
NT = N // 128        # 64 tiles of 128 embeddings
Q = 8                # row tiles per core
ARC = 33             # column tiles per row tile (o = 0..32)
OCOLS = 32 * 128     # 4096: o0..o31 region; o32 handled host-side
ZCOLS = (Q - 1 + 32 + 1) * 128   # 5120 rolled columns needed per core
TEMP = 0.07
ZSCALE = 16.0        # pre-scale before fp8 cast (keeps values in e4m3 normal range)
EXP_SCALE = 1.0 / (ZSCALE * ZSCALE * TEMP)

F32 = mybir.dt.float32
BF16 = mybir.dt.bfloat16
FP8 = mybir.dt.float8e4

# boundary col-sum chunks (partial per-q2 coverage; unwritten targets in
# the zero-marked bank stay 0, so the host adds the full 512 blindly)
TAIL_CHUNKS = [0, 1, 8, 9]

RS_PARTS = 4         # rowsum partials per q: [ACT-accum B0, DVE B1, B2, B3]

# z DMA column chunks, issued in ascending order across three engine
# sequencers so early gram matmuls only wait on the small first chunk.
ZCHUNKS = [(0, 1024), (1024, 2048), (2048, 3072), (3072, 4096), (4096, 5120)]


def build_nc() -> bass.Bass:
    nc = bacc.Bacc("TRN2", target_bir_lowering=False, debug=False, num_devices=NCORES)
    zdr = nc.declare_dram_parameter("zdr", [128, 2, ZCOLS], FP8, isOutput=False)
    ones_d = nc.declare_dram_parameter("ones", [128, 32], BF16, isOutput=False)
    rowsum_d = nc.declare_dram_parameter("rowsum", [128, RS_PARTS * Q], F32, isOutput=True)
    o32_d = nc.declare_dram_parameter("o32", [128, Q * 128], F32, isOutput=True)
    mid_d = nc.declare_dram_parameter("mid", [1, 6 * 512], F32, isOutput=True)
    edge_d = nc.declare_dram_parameter("edge", [1, 4 * 512], F32, isOutput=True)

    with tile.TileContext(nc) as tc:
        with (
            tc.tile_pool(name="zp", bufs=1) as zp,
            tc.tile_pool(name="const", bufs=1) as constp,
            tc.tile_pool(name="expp", bufs=8) as expp,
            tc.tile_pool(name="stats", bufs=1) as statsp,
            tc.tile_pool(name="gram", bufs=2, space="PSUM") as gramp,
            tc.tile_pool(name="cacc", bufs=1, space="PSUM") as caccp,
        ):
            # PE p-state warm-up: the tensor engine ramps to full clock only
            # after ~3us of CONTINUOUS execution (an idle gap resets the
            # ramp), so stream many short dummy matmuls over a zeroed tile
            # until the first z chunk lands -- the first real gram then
            # queues behind at most one ~100ns dummy and runs at full speed.
            # Both memsets go on the idle DVE: anything queued on ACT would
            # sit behind the 1.3us exp-table load.
            wz = statsp.tile([128, 512], BF16)
            nc.vector.memset(wz[:], 0.0)
            zeros = statsp.tile([128, 512], F32)
            nc.vector.memset(zeros[:], 0.0)
            # ACT exp-table preload on the zeroed tile, overlapping the DMA
            warm = statsp.tile([32, 2], F32)
            nc.scalar.activation(
                warm[:, 0:1], zeros[0:32, 0:1], mybir.ActivationFunctionType.Exp
            )
            wps = gramp.tile([128, 1024], F32, tag="g")
            for _ in range(30):
                nc.tensor.matmul(
                    wps[:, 0:128], lhsT=wz[:, 0:128], rhs=wz[:, 0:128],
                    start=True, stop=True,
                )


            # Spread the input DMA issue across idle engine sequencers: a
            # single sequencer spends ~620ns per dma_start dispatch, so 7
            # serial dispatches on sync would delay the last z chunk by 4us.
            z = zp.tile([128, 2, ZCOLS], FP8)
            ones = constp.tile([128, 32], BF16)
            issuers = [nc.sync, nc.scalar, nc.gpsimd, nc.sync, nc.scalar]
            for (c0, c1), eng in zip(ZCHUNKS, issuers):
                eng.dma_start(z[:, :, c0:c1], zdr[:, :, c0:c1])
            nc.sync.dma_start(ones[:], ones_d[:])

            rowsum = statsp.tile([128, RS_PARTS * Q], F32)
            scr = statsp.tile([128, 1024], BF16)
            o32stage = statsp.tile([128, Q * 128], F32)
            edgestage = statsp.tile([32, 4 * 512], F32)
            midstage = statsp.tile([32, 6 * 512], F32)
            # long-lived col-sum accumulator banks (chunks cc=2..5); one
            # tile per bank -- the tile framework serializes accumulation
            # groups per tile, so regions of a shared tile would ping-pong
            macc = [
                caccp.tile([128, 512], F32, tag=f"M{i}", name=f"macc{i}")
                for i in range(4)
            ]

            def gram_mm(out_ap, qs, c0, c1):
                """out = z[:, qs:qs+128].T @ z[:, c0:c1] (scaled x256)."""
                nc.tensor.matmul(
                    out_ap,
                    lhsT=z[:, :, qs:qs + 128],
                    rhs=z[:, :, c0:c1],
                    start=True,
                    stop=True,
                    perf_mode=mybir.MatmulPerfMode.DoubleRow,
                )

            def dve_copy(dst, src, eng=None):
                # PSUM -> SBUF drain; only one non-scalar input may be PSUM
                p = src.shape[0]
                w = src.shape[-1]
                (eng or nc.vector).scalar_tensor_tensor(
                    out=dst,
                    in0=src,
                    scalar=0.0,
                    in1=zeros[0:p, 0:w],
                    op0=mybir.AluOpType.bypass,
                    op1=mybir.AluOpType.add,
                )

            ets = []

            # et offset of arc column x (x = rolled col - qs): et_off = x
            def colsum(granule, tq, x0, w, start, stop):
                nc.tensor.matmul(
                    granule,
                    lhsT=ones[:],
                    rhs=ets[tq][:, x0:x0 + w],
                    start=start,
                    stop=stop,
                )

            def mid_colsum(q2, cc):
                """Chunk cc (target tiles 4cc..4cc+3): PE accumulates across
                q2 in an exclusive PSUM bank."""
                colsum(
                    macc[cc - 2][0:32, 0:512], q2, 512 * cc - 128 * q2, 512,
                    start=(q2 == 0), stop=(q2 == Q - 1),
                )

            def tail_chunk(cc, bank, boff):
                """Boundary col-sum chunk (global tiles 4cc..4cc+3) as ONE
                accumulation group: each q2 writes only its valid target
                sub-range (the group start zero-marks the whole bank row,
                so sub-range writes accumulate independently and unwritten
                targets stay zero). Replaces per-tile edge groups with a
                few wide matmuls and a single drain."""
                q2s = []
                for q2 in range(Q):
                    tlo = max(4 * cc, q2 + 1)
                    thi = min(4 * cc + 3, q2 + 31)
                    if tlo <= thi:
                        q2s.append((q2, tlo, thi))
                for i, (q2, tlo, thi) in enumerate(q2s):
                    os_ = 128 * tlo - 512 * cc
                    w = 128 * (thi - tlo + 1)
                    colsum(
                        bank[0:32, boff + os_:boff + os_ + w],
                        q2, 128 * (tlo - q2), w,
                        start=(i == 0), stop=(i == len(q2s) - 1),
                    )

            for q in range(Q):
                qs = 128 * q
                et = expp.tile([128, OCOLS], BF16, tag="et")
                ets.append(et)
                # gram blocks B0..B3: 1024-wide slabs, 2x512 matmuls each,
                # exp'd as soon as written; col-sum filler interleaved so
                # the PE never idles waiting on ACT to free a PSUM slot
                # col-sum fillers go BEFORE each gram pair on the PE queue:
                # ready work must not sit behind a gram that's waiting on an
                # exp to free its PSUM slot (in-order head-of-line), and a
                # busy PE stays out of the slow p-states.
                pb = []
                for b in range(2):
                    if q >= 1:
                        mid_colsum(q - 1, 2 + b)
                    p = gramp.tile([128, 1024], F32, tag="g")
                    base = qs + 1024 * b
                    gram_mm(p[:, 0:512], qs, base, base + 512)
                    gram_mm(p[:, 512:1024], qs, base + 512, base + 1024)
                    pb.append(p)
                # rowsum partials: B0 on the DVE (which has slack), B1 on
                # the otherwise-idle gpsimd via tensor_scalar's accumulator
                # (SBUF-only op, so the no-PSUM-on-gpsimd rule is fine) --
                # keeps the ACT queue free of accumulator reads entirely
                nc.scalar.activation(
                    et[:, 0:1024], pb[0][:],
                    mybir.ActivationFunctionType.Exp, scale=EXP_SCALE,
                )
                nc.vector.tensor_reduce(
                    out=rowsum[:, RS_PARTS * q:RS_PARTS * q + 1],
                    in_=et[:, 0:1024],
                    axis=mybir.AxisListType.X,
                    op=mybir.AluOpType.add,
                )
                nc.scalar.activation(
                    et[:, 1024:2048], pb[1][:],
                    mybir.ActivationFunctionType.Exp, scale=EXP_SCALE,
                )
                nc.gpsimd.tensor_tensor(
                    out=scr[:, 0:512],
                    in0=et[:, 1024:1536],
                    in1=et[:, 1536:2048],
                    op=mybir.AluOpType.add,
                )
                nc.gpsimd.tensor_tensor(
                    out=scr[:, 512:768],
                    in0=scr[:, 0:256],
                    in1=scr[:, 256:512],
                    op=mybir.AluOpType.add,
                )
                nc.vector.tensor_reduce(
                    out=rowsum[:, RS_PARTS * q + 1:RS_PARTS * q + 2],
                    in_=scr[:, 512:768],
                    axis=mybir.AxisListType.X,
                    op=mybir.AluOpType.add,
                )
                for b in range(2, 4):
                    if q >= 1:
                        mid_colsum(q - 1, 2 + b)
                    p = gramp.tile([128, 1024], F32, tag="g")
                    base = qs + 1024 * b
                    gram_mm(p[:, 0:512], qs, base, base + 512)
                    gram_mm(p[:, 512:1024], qs, base + 512, base + 1024)
                    nc.scalar.activation(
                        et[:, 1024 * b:1024 * (b + 1)], p[:],
                        mybir.ActivationFunctionType.Exp, scale=EXP_SCALE,
                    )
                for j, (x0, x1) in enumerate(
                    [(2048, 3072), (3072, 4096)], start=2
                ):
                    nc.vector.tensor_reduce(
                        out=rowsum[:, RS_PARTS * q + j:RS_PARTS * q + j + 1],
                        in_=et[:, x0:x1],
                        axis=mybir.AxisListType.X,
                        op=mybir.AluOpType.add,
                    )

            for cc in range(2, 6):
                mid_colsum(Q - 1, cc)
            # rowsum is complete once the loop's last reduces land
            nc.gpsimd.dma_start(rowsum_d[:], rowsum[:])

            # tail: cc=6,7 run through fresh gram-pool tiles (their banks
            # are free after the last exp) while one batched DVE op drains
            # all four macc banks; then the 8 o=32 gram blocks and 14 edge
            # tiles interleave over an 8-slot bank rotation (4 macc banks +
            # 2 spare banks in each cc tile), o32 drains on ACT, edge
            # drains alternating DVE/ACT, so the PE streams while both
            # drain engines trail behind
            # the raw o=32 gram blocks [qs+4096, qs+4224) go to the host,
            # which computes their exp row-sums and the target-pair diagonal
            # in fp64 -- no exp/diag work on device. They run FIRST in the
            # tail (their ACT copies overlap the cc=6,7 matmul stream, and
            # the 512KB o32 output DMA then overlaps the edge phase instead
            # of gating the kernel end).
            def o32_group(q2s, bank):
                """4 o=32 gram blocks share one accumulation group in one
                bank, drained by a single ACT copy."""
                for j, q2 in enumerate(q2s):
                    qs = 128 * q2
                    nc.tensor.matmul(
                        bank[:, 128 * j:128 * (j + 1)],
                        lhsT=z[:, :, qs:qs + 128],
                        rhs=z[:, :, qs + 4096:qs + 4224],
                        start=(j == 0), stop=(j == len(q2s) - 1),
                        perf_mode=mybir.MatmulPerfMode.DoubleRow,
                    )
                nc.scalar.copy(
                    o32stage[:, 128 * q2s[0]:128 * (q2s[-1] + 1)],
                    bank[:, 0:128 * len(q2s)],
                )

            gt6 = gramp.tile([128, 1024], F32, tag="g")
            for q2 in range(Q):
                colsum(
                    gt6[0:32, 0:512], q2, 512 * 6 - 128 * q2, 512,
                    start=(q2 == 0), stop=(q2 == 7),
                )
            # cc=1 rides gt6's spare second bank -- free at tail start
            tail_chunk(1, gt6, 512)
            for k in range(4):
                dve_copy(midstage[:, 512 * k:512 * (k + 1)], macc[k][0:32, 0:512])
            o32_group([0, 1, 2, 3], macc[0][:, 0:512])
            nc.gpsimd.dma_start(o32_d[:, 0:512], o32stage[:, 0:512])
            o32_group([4, 5, 6, 7], macc[1][:, 0:512])
            nc.gpsimd.dma_start(o32_d[:, 512:1024], o32stage[:, 512:1024])
            gt7 = gramp.tile([128, 1024], F32, tag="g")
            for q2 in range(Q):
                colsum(
                    gt7[0:32, 0:512], q2, 512 * 7 - 128 * q2, 512,
                    start=(q2 == 0), stop=(q2 == 7),
                )
            tail_chunk(0, gt7, 512)
            dve_copy(midstage[:, 2048:2560], gt6[0:32, 0:512])
            nc.scalar.copy(midstage[:, 2560:3072], gt7[0:32, 0:512])
            nc.gpsimd.dma_start(mid_d[:], midstage[0:1, :])
            dve_copy(edgestage[:, 512:1024], gt6[0:32, 512:1024])
            nc.scalar.copy(edgestage[:, 0:512], gt7[0:32, 512:1024])
            nc.sync.dma_start(edge_d[:, 0:1024], edgestage[0:1, 0:1024])
            # cc=8,9 through macc banks 2/3 (free after the mid drains)
            tail_chunk(8, macc[2], 0)
            tail_chunk(9, macc[3], 0)
            dve_copy(edgestage[:, 1024:1536], macc[2][0:32, 0:512])
            nc.scalar.copy(edgestage[:, 1536:2048], macc[3][0:32, 0:512])
            nc.sync.dma_start(edge_d[:, 1024:2048], edgestage[0:1, 1024:2048])

    nc.compile()
    return nc


def make_in_maps(z1: np.ndarray, z2: np.ndarray) -> list[dict]:
    z = np.concatenate([np.asarray(z1), np.asarray(z2)], axis=0).astype(np.float64)
    zn = z / np.maximum(np.linalg.norm(z, axis=1, keepdims=True), 1e-12)
    ztn = np.ascontiguousarray(zn.T * ZSCALE)  # [256, 8192]
    zdt = mybir.dt.np(FP8)
    onesm = np.ones((128, 32), dtype=mybir.dt.np(BF16))
    in_maps = []
    for c in range(NCORES):
        rolled = np.roll(ztn, -1024 * c, axis=1)[:, :ZCOLS]
        # DoubleRow layout: [partition p, ko, x] = row (128*ko + p)
        zdr = np.ascontiguousarray(
            rolled.reshape(2, 128, ZCOLS).transpose(1, 0, 2)
        ).astype(zdt)
        in_maps.append({"zdr": zdr, "ones": onesm})
    return in_maps


def assemble(results: list[dict]) -> np.float32:
    S = np.zeros(N, dtype=np.float64)
    tgt_all = np.zeros(N, dtype=np.float64)
    for c in range(NCORES):
        r = results[c]
        rowsum = (
            r["rowsum"].astype(np.float64).reshape(128, Q, RS_PARTS).sum(axis=-1)
        )
        # raw o=32 gram blocks [p, q*128+j]: exp row-sums and the
        # target-pair diagonal both come out in fp64 here
        o32 = r["o32"].astype(np.float64).reshape(128, Q, 128)
        rowsum += np.exp(o32 * EXP_SCALE).sum(axis=-1)
        tgt = o32[np.arange(128), :, np.arange(128)]  # [128, Q]
        mid = r["mid"].astype(np.float64).reshape(6, 512)
        edge = r["edge"].astype(np.float64).reshape(4, 512)
        base = 1024 * c
        for q in range(Q):
            S[base + 128 * q: base + 128 * (q + 1)] += rowsum[:, q]
            tgt_all[base + 128 * q: base + 128 * (q + 1)] = tgt[:, q]
        for cc in range(2, 8):
            gidx = (512 * cc + np.arange(512) + base) % N
            S[gidx] += mid[cc - 2]
        for k, cck in enumerate(TAIL_CHUNKS):
            gidx = (512 * cck + np.arange(512) + base) % N
            S[gidx] += edge[k]
    loss = np.mean(np.log(S) - tgt_all / (ZSCALE * ZSCALE * TEMP))
    return np.float32(loss)


_NC_CACHE: list = []


def kernel(z1: np.ndarray, z2: np.ndarray) -> np.ndarray:
    in_maps = make_in_maps(z1, z2)
    if not _NC_CACHE:
        _NC_CACHE.append(build_nc())
    nc = _NC_CACHE[0]
    res = run_bass_kernel_spmd(nc, in_maps, list(range(NCORES)))
    return assemble(res.results)


if __name__ == "__main__":
    rng = np.random.default_rng(0)
    z1 = rng.standard_normal((B, D), dtype=np.float32)
    z2 = rng.standard_normal((B, D), dtype=np.float32)
    print(kernel(z1, z2))


# revision 86
# speedup vs baseline: 1.4551x; 1.0026x over previous
"""NT-Xent contrastive loss (SimCLR-style) on 8 Trainium2 NeuronCores.

Problem: z1, z2 [4096, 256] fp32 -> scalar loss.
  zn = l2norm(z), z = concat(z1, z2) -> [8192, 256]
  sim = zn @ zn.T / 0.07              -> [8192, 8192]
  loss = -mean_i log_softmax(sim)[i, partner(i)],  partner(i) = (i + 4096) % 8192

Strategy (symmetric): exp(sim) is symmetric, so each unordered tile pair
{a, b} of the 64x64 grid of 128x128 blocks is computed ONCE. The core
owning row-tile a computes blocks (a, a+o mod 64) for o = 0..32 (the
o=32 pair is computed by both owners: 3% redundancy that keeps the
program SPMD-identical). The transpose credit for o = 1..31 comes from
COLUMN sums of the exp blocks, computed on the PE as ones^T @ E matmuls
accumulating in PSUM. Host adds the per-core partial sums.

Per-core input is the normalized z^T rolled so its own 1024 rows sit at
columns [0, 1024): every core runs one identical program, and the o-arcs
become contiguous column ranges [0, 5120) -- only 62.5% of z is even
loaded. Matmuls run in fp8e4m3 (values pre-scaled x16) with DoubleRow
packing K=256 into one pass; exp runs on the scalar (ACT) engine
PSUM->SBUF in bf16. Tolerance is rel 2e-2; fp8 error lands ~1e-3.

The steady-state loop is ACT-bound (4x 1024-wide exp per row tile,
~4.0us) and runs with ZERO gaps on the ACT queue -- pure back-to-back
exps, no accumulator reads. That requires exactly 4 gram-block writes
and 4 exp reads per q through the 2 PSUM gram slots -- the classic
double buffer where each write lands while the other slot's exp
executes. Everything that would break the 4-allocation rhythm is
evicted from the loop:
  - rowsum: B0/B2/B3 are DVE tensor_reduce partials (the DVE
    reduce-with-accumulate ISA paths all run 1 el/cycle on hw, so a
    monolithic 4096-wide reduce would serialize everything); B1 folds
    1024->256 with two elementwise adds on the otherwise-idle gpsimd
    (Pool supports SBUF tensor_tensor but not reduce-accumulate ops)
    before a short DVE reduce; host adds the 4 partials.
  - the o=32 tile: its RAW gram block is computed at the tail, drained
    to SBUF, and shipped to the host, which computes both its exp
    row-sums and the target-pair diagonal (log-softmax numerator) in
    fp64. No exp, no diag mask, no 5th activation on device.
  - edge-tile col-sums (t=1..7 and 32..38, the partial-overlap chunks)
    all run at the tail.
exp SBUF layout per q: [o0 | ... | o31] (4096 cols), 1024-aligned with
both the PSUM banks and the z DMA chunks, which are issued in ascending
column order across three engine sequencers (one sequencer needs ~620ns
per dma_start dispatch, so serial issue would delay the last chunk 4us).
Col-sum matmuls are emitted BEFORE each gram pair: ready PE work must
not queue behind a gram that waits on an exp (in-order head-of-line),
and a busy PE stays out of the slow p-states. Dummy warm-up matmuls
during the input DMA ramp the PE clock (full speed needs ~3us of
continuous execution).

PSUM discipline: an accumulation group conflicts with any other group
in the same bank while it is live, and the tile framework serializes
groups per TILE, so long-lived accumulators get exclusive single-bank
tiles. Banks 0-3: gram double-buffer ([128,1024] x 2). Banks 4-7:
col-sum chunks cc=2..5 (target tiles 8..23), held open across the whole
q loop. At the tail, chunks cc=6,7 run through fresh gram-pool tiles,
then the 8 o=32 blocks and 14 edge tiles interleave over a 6-slot bank
rotation with drains split across DVE and ACT; edge tiles go largest-
first so the final output DMA is gated only by the cheapest drains.
"""

import numpy as np

import concourse.bass as bass
import concourse.tile as tile
from concourse import bacc, mybir
from concourse.bass_utils import run_bass_kernel_spmd

B, D = 4096, 256
N = 2 * B            # 8192 embeddings
NCORES = 8
NT = N // 128        # 64 tiles of 128 embeddings
Q = 8                # row tiles per core
ARC = 33             # column tiles per row tile (o = 0..32)
OCOLS = 32 * 128     # 4096: o0..o31 region; o32 handled host-side
ZCOLS = (Q - 1 + 32 + 1) * 128   # 5120 rolled columns needed per core
TEMP = 0.07
ZSCALE = 16.0        # pre-scale before fp8 cast (keeps values in e4m3 normal range)
EXP_SCALE = 1.0 / (ZSCALE * ZSCALE * TEMP)

F32 = mybir.dt.float32
BF16 = mybir.dt.bfloat16
FP8 = mybir.dt.float8e4

# boundary col-sum chunks (partial per-q2 coverage; unwritten targets in
# the zero-marked bank stay 0, so the host adds the full 512 blindly)
TAIL_CHUNKS = [0, 1, 8, 9]

RS_PARTS = 4         # rowsum partials per q: [ACT-accum B0, DVE B1, B2, B3]

# z DMA column chunks, issued in ascending order across three engine
# sequencers so early gram matmuls only wait on the small first chunk.
ZCHUNKS = [(0, 1024), (1024, 2048), (2048, 3072), (3072, 4096), (4096, 5120)]


def build_nc() -> bass.Bass:
    nc = bacc.Bacc("TRN2", target_bir_lowering=False, debug=False, num_devices=NCORES)
    zdr = nc.declare_dram_parameter("zdr", [128, 2, ZCOLS], FP8, isOutput=False)
    ones_d = nc.declare_dram_parameter("ones", [128, 32], BF16, isOutput=False)
    rowsum_d = nc.declare_dram_parameter("rowsum", [128, RS_PARTS * Q], F32, isOutput=True)
    o32_d = nc.declare_dram_parameter("o32", [128, Q * 128], F32, isOutput=True)
    mid_d = nc.declare_dram_parameter("mid", [1, 6 * 512], F32, isOutput=True)
    edge_d = nc.declare_dram_parameter("edge", [1, 4 * 512], F32, isOutput=True)

    with tile.TileContext(nc) as tc:
        with (
            tc.tile_pool(name="zp", bufs=1) as zp,
            tc.tile_pool(name="const", bufs=1) as constp,
            tc.tile_pool(name="expp", bufs=8) as expp,
            tc.tile_pool(name="stats", bufs=1) as statsp,
            tc.tile_pool(name="gram", bufs=2, space="PSUM") as gramp,
            tc.tile_pool(name="cacc", bufs=1, space="PSUM") as caccp,
        ):
            # PE p-state warm-up: the tensor engine ramps to full clock only
            # after ~3us of CONTINUOUS execution (an idle gap resets the
            # ramp), so stream many short dummy matmuls over a zeroed tile
            # until the first z chunk lands -- the first real gram then
            # queues behind at most one ~100ns dummy and runs at full speed.
            # Both memsets go on the idle DVE: anything queued on ACT would
            # sit behind the 1.3us exp-table load.
            wz = statsp.tile([128, 512], BF16)
            nc.vector.memset(wz[:], 0.0)
            zeros = statsp.tile([128, 512], F32)
            nc.vector.memset(zeros[:], 0.0)
            # ACT exp-table preload on the zeroed tile, overlapping the DMA
            warm = statsp.tile([32, 2], F32)
            nc.scalar.activation(
                warm[:, 0:1], zeros[0:32, 0:1], mybir.ActivationFunctionType.Exp
            )
            wps = gramp.tile([128, 1024], F32, tag="g")
            for _ in range(30):
                nc.tensor.matmul(
                    wps[:, 0:128], lhsT=wz[:, 0:128], rhs=wz[:, 0:128],
                    start=True, stop=True,
                )


            # Spread the input DMA issue across idle engine sequencers: a
            # single sequencer spends ~620ns per dma_start dispatch, so 7
            # serial dispatches on sync would delay the last z chunk by 4us.
            z = zp.tile([128, 2, ZCOLS], FP8)
            ones = constp.tile([128, 32], BF16)
            issuers = [nc.sync, nc.scalar, nc.gpsimd, nc.sync, nc.scalar]
            for (c0, c1), eng in zip(ZCHUNKS, issuers):
                eng.dma_start(z[:, :, c0:c1], zdr[:, :, c0:c1])
            nc.sync.dma_start(ones[:], ones_d[:])

            rowsum = statsp.tile([128, RS_PARTS * Q], F32)
            scr = statsp.tile([128, 1024], BF16)
            o32stage = statsp.tile([128, Q * 128], F32)
            edgestage = statsp.tile([32, 4 * 512], F32)
            midstage = statsp.tile([32, 6 * 512], F32)
            # long-lived col-sum accumulator banks (chunks cc=2..5); one
            # tile per bank -- the tile framework serializes accumulation
            # groups per tile, so regions of a shared tile would ping-pong
            macc = [
                caccp.tile([128, 512], F32, tag=f"M{i}", name=f"macc{i}")
                for i in range(4)
            ]

            def gram_mm(out_ap, qs, c0, c1):
                """out = z[:, qs:qs+128].T @ z[:, c0:c1] (scaled x256)."""
                nc.tensor.matmul(
                    out_ap,
                    lhsT=z[:, :, qs:qs + 128],
                    rhs=z[:, :, c0:c1],
                    start=True,
                    stop=True,
                    perf_mode=mybir.MatmulPerfMode.DoubleRow,
                )

            def dve_copy(dst, src, eng=None):
                # PSUM -> SBUF drain; only one non-scalar input may be PSUM
                p = src.shape[0]
                w = src.shape[-1]
                (eng or nc.vector).scalar_tensor_tensor(
                    out=dst,
                    in0=src,
                    scalar=0.0,
                    in1=zeros[0:p, 0:w],
                    op0=mybir.AluOpType.bypass,
                    op1=mybir.AluOpType.add,
                )

            ets = []

            # et offset of arc column x (x = rolled col - qs): et_off = x
            def colsum(granule, tq, x0, w, start, stop):
                nc.tensor.matmul(
                    granule,
                    lhsT=ones[:],
                    rhs=ets[tq][:, x0:x0 + w],
                    start=start,
                    stop=stop,
                )

            def mid_colsum(q2, cc):
                """Chunk cc (target tiles 4cc..4cc+3): PE accumulates across
                q2 in an exclusive PSUM bank."""
                colsum(
                    macc[cc - 2][0:32, 0:512], q2, 512 * cc - 128 * q2, 512,
                    start=(q2 == 0), stop=(q2 == Q - 1),
                )

            def tail_chunk(cc, bank, boff):
                """Boundary col-sum chunk (global tiles 4cc..4cc+3) as ONE
                accumulation group: each q2 writes only its valid target
                sub-range (the group start zero-marks the whole bank row,
                so sub-range writes accumulate independently and unwritten
                targets stay zero). Replaces per-tile edge groups with a
                few wide matmuls and a single drain."""
                q2s = []
                for q2 in range(Q):
                    tlo = max(4 * cc, q2 + 1)
                    thi = min(4 * cc + 3, q2 + 31)
                    if tlo <= thi:
                        q2s.append((q2, tlo, thi))
                for i, (q2, tlo, thi) in enumerate(q2s):
                    os_ = 128 * tlo - 512 * cc
                    w = 128 * (thi - tlo + 1)
                    colsum(
                        bank[0:32, boff + os_:boff + os_ + w],
                        q2, 128 * (tlo - q2), w,
                        start=(i == 0), stop=(i == len(q2s) - 1),
                    )

            for q in range(Q):
                qs = 128 * q
                et = expp.tile([128, OCOLS], BF16, tag="et")
                ets.append(et)
                # gram blocks B0..B3: 1024-wide slabs, 2x512 matmuls each,
                # exp'd as soon as written; col-sum filler interleaved so
                # the PE never idles waiting on ACT to free a PSUM slot
                # col-sum fillers go BEFORE each gram pair on the PE queue:
                # ready work must not sit behind a gram that's waiting on an
                # exp to free its PSUM slot (in-order head-of-line), and a
                # busy PE stays out of the slow p-states.
                pb = []
                for b in range(2):
                    if q >= 1:
                        mid_colsum(q - 1, 2 + b)
                    p = gramp.tile([128, 1024], F32, tag="g")
                    base = qs + 1024 * b
                    gram_mm(p[:, 0:512], qs, base, base + 512)
                    gram_mm(p[:, 512:1024], qs, base + 512, base + 1024)
                    pb.append(p)
                # rowsum partials: B0 on the DVE (which has slack), B1 on
                # the otherwise-idle gpsimd via tensor_scalar's accumulator
                # (SBUF-only op, so the no-PSUM-on-gpsimd rule is fine) --
                # keeps the ACT queue free of accumulator reads entirely
                nc.scalar.activation(
                    et[:, 0:1024], pb[0][:],
                    mybir.ActivationFunctionType.Exp, scale=EXP_SCALE,
                )
                nc.vector.tensor_reduce(
                    out=rowsum[:, RS_PARTS * q:RS_PARTS * q + 1],
                    in_=et[:, 0:1024],
                    axis=mybir.AxisListType.X,
                    op=mybir.AluOpType.add,
                )
                nc.scalar.activation(
                    et[:, 1024:2048], pb[1][:],
                    mybir.ActivationFunctionType.Exp, scale=EXP_SCALE,
                )
                nc.gpsimd.tensor_tensor(
                    out=scr[:, 0:512],
                    in0=et[:, 1024:1536],
                    in1=et[:, 1536:2048],
                    op=mybir.AluOpType.add,
                )
                nc.gpsimd.tensor_tensor(
                    out=scr[:, 512:768],
                    in0=scr[:, 0:256],
                    in1=scr[:, 256:512],
                    op=mybir.AluOpType.add,
                )
                nc.vector.tensor_reduce(
                    out=rowsum[:, RS_PARTS * q + 1:RS_PARTS * q + 2],
                    in_=scr[:, 512:768],
                    axis=mybir.AxisListType.X,
                    op=mybir.AluOpType.add,
                )
                if q == 0:
                    # the raw o=32 gram blocks go to the host (it computes
                    # their exp row-sums + the target diagonal in fp64).
                    # They run DURING q0: the macc banks are untouched until
                    # cc2/cc3's first contribution mid-q1, the PE is idle
                    # waiting on exp(B0/B1) anyway, the drains fill the
                    # DVE's empty q0 window, and the 512KB output DMA ships
                    # ~35us before the kernel end instead of gating it.
                    for half in range(2):
                        bank = macc[half][:, 0:512]
                        for j in range(4):
                            qs = 128 * (4 * half + j)
                            nc.tensor.matmul(
                                bank[:, 128 * j:128 * (j + 1)],
                                lhsT=z[:, :, qs:qs + 128],
                                rhs=z[:, :, qs + 4096:qs + 4224],
                                start=(j == 0), stop=(j == 3),
                                perf_mode=mybir.MatmulPerfMode.DoubleRow,
                            )
                        dve_copy(
                            o32stage[:, 512 * half:512 * (half + 1)], bank
                        )
                    nc.gpsimd.dma_start(o32_d[:], o32stage[:])
                for b in range(2, 4):
                    if q >= 1:
                        mid_colsum(q - 1, 2 + b)
                    p = gramp.tile([128, 1024], F32, tag="g")
                    base = qs + 1024 * b
                    gram_mm(p[:, 0:512], qs, base, base + 512)
                    gram_mm(p[:, 512:1024], qs, base + 512, base + 1024)
                    nc.scalar.activation(
                        et[:, 1024 * b:1024 * (b + 1)], p[:],
                        mybir.ActivationFunctionType.Exp, scale=EXP_SCALE,
                    )
                for j, (x0, x1) in enumerate(
                    [(2048, 3072), (3072, 4096)], start=2
                ):
                    nc.vector.tensor_reduce(
                        out=rowsum[:, RS_PARTS * q + j:RS_PARTS * q + j + 1],
                        in_=et[:, x0:x1],
                        axis=mybir.AxisListType.X,
                        op=mybir.AluOpType.add,
                    )

            for cc in range(2, 6):
                mid_colsum(Q - 1, cc)
            # rowsum is complete once the loop's last reduces land
            nc.gpsimd.dma_start(rowsum_d[:], rowsum[:])

            # tail: 10 col-sum chunk drains total. cc=6,7 run through fresh
            # gram-pool tiles (their banks free after the last exp), the
            # boundary chunks cc=1,0 ride those tiles' spare second banks,
            # and cc=8,9 reuse macc2/3 after the mid drains; drains split
            # across DVE and ACT so the PE streams while both trail behind
            gt6 = gramp.tile([128, 1024], F32, tag="g")
            for q2 in range(Q):
                colsum(
                    gt6[0:32, 0:512], q2, 512 * 6 - 128 * q2, 512,
                    start=(q2 == 0), stop=(q2 == 7),
                )
            # cc=1 rides gt6's spare second bank -- free at tail start
            tail_chunk(1, gt6, 512)
            for k in range(4):
                dve_copy(midstage[:, 512 * k:512 * (k + 1)], macc[k][0:32, 0:512])
            gt7 = gramp.tile([128, 1024], F32, tag="g")
            for q2 in range(Q):
                colsum(
                    gt7[0:32, 0:512], q2, 512 * 7 - 128 * q2, 512,
                    start=(q2 == 0), stop=(q2 == 7),
                )
            tail_chunk(0, gt7, 512)
            dve_copy(midstage[:, 2048:2560], gt6[0:32, 0:512])
            nc.scalar.copy(midstage[:, 2560:3072], gt7[0:32, 0:512])
            nc.gpsimd.dma_start(mid_d[:], midstage[0:1, :])
            dve_copy(edgestage[:, 512:1024], gt6[0:32, 512:1024])
            nc.scalar.copy(edgestage[:, 0:512], gt7[0:32, 512:1024])
            nc.sync.dma_start(edge_d[:, 0:1024], edgestage[0:1, 0:1024])
            # cc=8,9 through macc banks 2/3 (free after the mid drains)
            tail_chunk(8, macc[2], 0)
            tail_chunk(9, macc[3], 0)
            dve_copy(edgestage[:, 1024:1536], macc[2][0:32, 0:512])
            nc.scalar.copy(edgestage[:, 1536:2048], macc[3][0:32, 0:512])
            nc.sync.dma_start(edge_d[:, 1024:2048], edgestage[0:1, 1024:2048])

    nc.compile()
    return nc


def make_in_maps(z1: np.ndarray, z2: np.ndarray) -> list[dict]:
    z = np.concatenate([np.asarray(z1), np.asarray(z2)], axis=0).astype(np.float64)
    zn = z / np.maximum(np.linalg.norm(z, axis=1, keepdims=True), 1e-12)
    ztn = np.ascontiguousarray(zn.T * ZSCALE)  # [256, 8192]
    zdt = mybir.dt.np(FP8)
    onesm = np.ones((128, 32), dtype=mybir.dt.np(BF16))
    in_maps = []
    for c in range(NCORES):
        rolled = np.roll(ztn, -1024 * c, axis=1)[:, :ZCOLS]
        # DoubleRow layout: [partition p, ko, x] = row (128*ko + p)
        zdr = np.ascontiguousarray(
            rolled.reshape(2, 128, ZCOLS).transpose(1, 0, 2)
        ).astype(zdt)
        in_maps.append({"zdr": zdr, "ones": onesm})
    return in_maps


def assemble(results: list[dict]) -> np.float32:
    S = np.zeros(N, dtype=np.float64)
    tgt_all = np.zeros(N, dtype=np.float64)
    for c in range(NCORES):
        r = results[c]
        rowsum = (
            r["rowsum"].astype(np.float64).reshape(128, Q, RS_PARTS).sum(axis=-1)
        )
        # raw o=32 gram blocks [p, q*128+j]: exp row-sums and the
        # target-pair diagonal both come out in fp64 here
        o32 = r["o32"].astype(np.float64).reshape(128, Q, 128)
        rowsum += np.exp(o32 * EXP_SCALE).sum(axis=-1)
        tgt = o32[np.arange(128), :, np.arange(128)]  # [128, Q]
        mid = r["mid"].astype(np.float64).reshape(6, 512)
        edge = r["edge"].astype(np.float64).reshape(4, 512)
        base = 1024 * c
        for q in range(Q):
            S[base + 128 * q: base + 128 * (q + 1)] += rowsum[:, q]
            tgt_all[base + 128 * q: base + 128 * (q + 1)] = tgt[:, q]
        for cc in range(2, 8):
            gidx = (512 * cc + np.arange(512) + base) % N
            S[gidx] += mid[cc - 2]
        for k, cck in enumerate(TAIL_CHUNKS):
            gidx = (512 * cck + np.arange(512) + base) % N
            S[gidx] += edge[k]
    loss = np.mean(np.log(S) - tgt_all / (ZSCALE * ZSCALE * TEMP))
    return np.float32(loss)


_NC_CACHE: list = []


def kernel(z1: np.ndarray, z2: np.ndarray) -> np.ndarray:
    in_maps = make_in_maps(z1, z2)
    if not _NC_CACHE:
        _NC_CACHE.append(build_nc())
    nc = _NC_CACHE[0]
    res = run_bass_kernel_spmd(nc, in_maps, list(range(NCORES)))
    return assemble(res.results)


if __name__ == "__main__":
    rng = np.random.default_rng(0)
    z1 = rng.standard_normal((B, D), dtype=np.float32)
    z2 = rng.standard_normal((B, D), dtype=np.float32)
    print(kernel(z1, z2))
